# revision 1
# baseline (speedup 1.0000x reference)
"""DisMax loss first part: logits = -(|s|*d + mean_c(|s|*d)) / temp, where
d[b,c] = ||fn_b - pn_c|| / sqrt(2) = sqrt(1 - cos(f_b, p_c)) for l2-normalized rows.

Strategy: data-parallel over the batch across 8 NeuronCores. Each core:
  [1024, 512] features x [512, 10000] prototypes -> [1024, 10000].
Both operands arrive host-transposed ([d, .] layout prep only); the device
computes all numerics:
  - norms via PE Gram blocks: for each 128-col block, matmul(block, block)
    in f32r accumulates X^T X into PSUM; one DVE tensor_tensor_reduce with
    an identity mask extracts the diagonal (col sums of squares) - no
    elementwise square pass over the 20 MB of prototypes;
  - ACT Sqrt on the diag columns (same act table as the main-loop Sqrt, so
    no table reloads), PE transpose + SBUF->SBUF DMA flatten to a [1, W]
    norm row, PE ones-matmul broadcast to 128 partitions, DVE divide
    producing fp8e4 GEMM operands;
  - main GEMM in fp8e4 DoubleRow perf mode (2 contraction sub-tiles per
    pass, 2x PE throughput), fp32 PSUM; ACT computes sqrt(1 - cos) out of
    PSUM with fused row-sum accumulation, writing bf16 distances;
  - GPSIMD applies out = dist*c0 + rowsum*c1 (c0 = -|scale|/temp,
    c1 = c0/10000) in bf16; the 20 MB bf16 result streams to HBM and the
    host upcasts to f32 (rel err from fp8 GEMM + bf16 out ~5e-3 << 2e-2).
Batch tile 0's GEMM chunks are issued interleaved with prototype prep so
PE/ACT/DMA pipeline from t~=10us instead of serializing prep before main.
"""

import sys
import types

for _p in ("/opt/trn_rl_repo", "/root/.axon_site"):
    if _p not in sys.path:
        sys.path.insert(0, _p)

# The NTFF profiling hook module is absent from this image's antenv package;
# inject the ctypes-based equivalent so trace=True works when requested.
if "antenv.axon_hooks" not in sys.modules:
    try:
        import trn_agent_boot.trn_boot as _tb

        _hook = _tb._ntff_profile_via_ctypes("/opt/axon/libaxon_pjrt.so")
        _m = types.ModuleType("antenv.axon_hooks")
        _m.get_axon_ntff_profile_hook = lambda: _hook
        sys.modules["antenv.axon_hooks"] = _m
    except Exception:
        pass

import numpy as np

import concourse.bacc as bacc
import concourse.tile as tile
import concourse.mybir as mybir
from concourse.bass_utils import run_bass_kernel_spmd

F32 = mybir.dt.float32
F32R = mybir.dt.float32r
BF16 = mybir.dt.bfloat16
FP8 = mybir.dt.float8e4
ALU = mybir.AluOpType
ACTF = mybir.ActivationFunctionType
DR = mybir.MatmulPerfMode.DoubleRow

N_CORES = 8
B, C, D = 8192, 10000, 512
BPC = B // N_CORES          # 1024 batch rows per core
NB = BPC // 128             # 8 batch tiles
ND = D // 128               # 4 contraction sub-tiles
CCH = 500                   # PSUM bank chunk (f32)
BCH = 1000                  # prototype prep chunk / ACT span
NBCH = C // BCH             # 10
P2 = 2000                   # store chunk (512 KB bf16 DMA)
NP2 = C // P2               # 5


def build_nc():
    nc = bacc.Bacc("TRN2", target_bir_lowering=False, debug=False,
                   num_devices=N_CORES)
    ft_h = nc.dram_tensor("ft", [D, BPC], F32, kind="ExternalInput")
    pt_h = nc.dram_tensor("pt", [D, C], F32, kind="ExternalInput")
    s_h = nc.dram_tensor("s", [1, 2], F32, kind="ExternalInput")
    o_h = nc.dram_tensor("o", [BPC, C], BF16, kind="ExternalOutput")

    from contextlib import ExitStack

    with tile.TileContext(nc) as tc:
        with ExitStack() as stack:
            ep = stack.enter_context
            const_pool = ep(tc.tile_pool(name="const", bufs=1))
            persist_pool = ep(tc.tile_pool(name="persist", bufs=1))
            fstage_pool = ep(tc.tile_pool(name="fstage", bufs=1))
            pstage_pool = ep(tc.tile_pool(name="pstage", bufs=3))
            scr_pool = ep(tc.tile_pool(name="scr", bufs=2))
            dcol_pool = ep(tc.tile_pool(name="dcol", bufs=2))
            ncr_pool = ep(tc.tile_pool(name="ncr", bufs=2))
            nrow_pool = ep(tc.tile_pool(name="nrow", bufs=2))
            dq_pool = ep(tc.tile_pool(name="dq", bufs=2))
            rs_pool = ep(tc.tile_pool(name="rs", bufs=2))
            norm_pool = ep(tc.tile_pool(name="norms", bufs=2))
            ob_pool = ep(tc.tile_pool(name="ob", bufs=4))
            psum_c_pool = ep(tc.tile_pool(name="ps_c", bufs=2, space="PSUM"))
            psum_n_pool = ep(tc.tile_pool(name="ps_n", bufs=1, space="PSUM"))
            psum_g_pool = ep(tc.tile_pool(name="ps_g", bufs=2, space="PSUM"))

            # persistent fp8 transposed, normalized operands
            pnT = persist_pool.tile([128, ND, C], FP8, tag="pnT")
            fnT = persist_pool.tile([128, ND, BPC], FP8, tag="fnT")
            cb = persist_pool.tile([128, 2], F32, tag="cb")  # c0, c1

            ones_f = const_pool.tile([1, 128], F32, tag="ones_f")
            nc.vector.memset(ones_f[:, :], 1.0)
            ones_r = const_pool.tile([1, 128], F32R, tag="ones_r")
            nc.vector.tensor_copy(ones_r[:, :], ones_f[:, :])
            from concourse import masks

            # [128, 256] f32: identity in cols 0..127, zeros elsewhere
            idmask = const_pool.tile([128, 256], F32, tag="idmask")
            nc.vector.memset(idmask[:, :], 0.0)
            masks.make_identity(nc, idmask[:, 0:128], nomemset=True)
            identF = const_pool.tile([128, 128], F32, tag="identF")
            masks.make_identity(nc, identF[:, :])

            # ---- scalar params: c0 = -|ds|/temp, c1 = c0/C ----------------
            stile = const_pool.tile([1, 2], F32, tag="stile")
            nc.sync.dma_start(stile[:, :], s_h[:, :])
            cv = const_pool.tile([1, 2], F32, tag="cvals")
            tmp = const_pool.tile([1, 2], F32, tag="scaltmp")
            nc.scalar.activation(tmp[:, 0:1], stile[:, 0:1], ACTF.Abs)
            nc.vector.reciprocal(tmp[:, 1:2], stile[:, 1:2])
            nc.vector.scalar_tensor_tensor(cv[:, 0:1], tmp[:, 0:1], -1.0,
                                           tmp[:, 1:2], op0=ALU.mult,
                                           op1=ALU.mult)
            nc.vector.tensor_scalar(cv[:, 1:2], cv[:, 0:1], 1.0 / C, None,
                                    op0=ALU.mult)
            ps_b = psum_n_pool.tile([128, 2, 512], F32, tag="nb", name="cbb")
            nc.tensor.matmul(ps_b[:, 0, :2], ones_f[:, :], cv[:, :],
                             start=True, stop=True)
            nc.vector.tensor_copy(cb[:, :], ps_b[:, 0, :2])

            # ---- operand prep: seg 0 = features, segs 1..10 = prototypes --
            ft_r = ft_h[:, :].rearrange("(t p) b -> p t b", p=128)
            pt_r = pt_h[:, :].rearrange("(t p) c -> p t c", p=128)

            def prep_segment(s):
                if s == 0:
                    W = BPC
                    pst = fstage_pool.tile([128, ND, BPC], F32R, tag="fst")
                    nc.sync.dma_start(pst[:, :, :],
                                      ft_r[:, :, :].bitcast(F32R))
                else:
                    W = BCH
                    c_lo = (s - 1) * BCH
                    pst = pstage_pool.tile([128, ND, BCH], F32R, tag="pst",
                                           name=f"pst_{s}")
                    nc.sync.dma_start(pst[:, :, :],
                                      pt_r[:, :, c_lo:c_lo + BCH].bitcast(F32R))
                nblk = (W + 127) // 128
                # Gram-diag: col sums of squares without a square pass
                dcol = dcol_pool.tile([128, 8], F32, tag="dcol",
                                      name=f"dcol_{s}")
                # last block covers <128 classes; init so the full-tile
                # sqrt/reciprocal/transpose read defined data everywhere
                nc.vector.memset(dcol[:, :], 1.0)
                for b in range(nblk):
                    c0 = b * 128
                    bs = min(128, W - c0)
                    w2 = min(256, W - c0)
                    gm = psum_g_pool.tile([128, 256], F32, tag="gm",
                                          name=f"gm_{s}_{b}")
                    for d in range(ND):
                        nc.tensor.matmul(
                            gm[:bs, :w2],
                            pst[:, d, c0:c0 + bs],
                            pst[:, d, c0:c0 + w2],
                            start=(d == 0), stop=(d == ND - 1))
                    scr = scr_pool.tile([128, 256], BF16, tag="scr",
                                        name=f"scr_{s}_{b}")
                    nc.vector.scalar_tensor_tensor(
                        scr[:bs, :w2], gm[:bs, :w2], 1.0,
                        idmask[:bs, :w2], op0=ALU.mult, op1=ALU.mult,
                        accum_out=dcol[:bs, b:b + 1])
                # norms = sqrt(diag); to [1, W] row via transpose + DMA
                ncol = dcol_pool.tile([128, 8], F32, tag="ncol",
                                      name=f"ncol_{s}")
                nc.scalar.activation(ncol[:, :nblk], dcol[:, :nblk],
                                     ACTF.Sqrt)
                # inverse norms (DVE reciprocal; tiny) so normalize is a mult
                nc.vector.reciprocal(ncol[:, :nblk], ncol[:, :nblk])
                tps = psum_g_pool.tile([128, 256], F32, tag="gm",
                                       name=f"tp_{s}")
                nc.tensor.transpose(tps[:nblk, :128], ncol[:, :nblk],
                                    identF[:, :])
                ncr = ncr_pool.tile([8, 128], F32, tag="ncr",
                                    name=f"ncr_{s}")
                nc.vector.tensor_copy(ncr[:nblk, :], tps[:nblk, :128])
                nrow = nrow_pool.tile([1, 1024], F32R, tag="nrow",
                                      name=f"nrow_{s}")
                nc.sync.dma_start(nrow[:, :nblk * 128],
                                  ncr[:nblk, :].bitcast(F32R))
                # broadcast norms to 128 partitions; normalize to fp8
                wb = W // 2
                nb = psum_n_pool.tile([128, 2, 512], F32, tag="nb",
                                      name=f"nb_{s}")
                for h in range(2):
                    nc.tensor.matmul(nb[:, h, :wb],
                                     ones_r[:, :],
                                     nrow[:, h * wb:(h + 1) * wb],
                                     start=True, stop=True)
                dst = fnT if s == 0 else pnT
                off = 0 if s == 0 else (s - 1) * BCH
                for d in range(ND):
                    nc.vector.tensor_tensor(
                        dst[:, d, off:off + W].rearrange(
                            "p (h c) -> p h c", h=2),
                        pst[:, d, :].bitcast(F32).rearrange(
                            "p (h c) -> p h c", h=2),
                        nb[:, :, :wb], op=ALU.mult)

            # ---- main loop: one 128-row batch tile ------------------------
            def main_chunk(i, cc, rs, dq):
                """GEMM + sqrt for classes [cc*1000, (cc+1)*1000), tile i."""
                pc = psum_c_pool.tile([128, 2, 512], F32, tag="pc",
                                      name=f"pc_{i}_{cc}")
                for h in range(2):
                    c0 = cc * BCH + h * CCH
                    for dp in range(ND // 2):
                        nc.tensor.matmul(
                            pc[:, h, :CCH],
                            fnT[:, 2 * dp:2 * dp + 2,
                                i * 128:(i + 1) * 128],
                            pnT[:, 2 * dp:2 * dp + 2, c0:c0 + CCH],
                            start=(dp == 0), stop=(dp == ND // 2 - 1),
                            perf_mode=DR)
                dv = dq[:, cc * BCH:(cc + 1) * BCH].rearrange(
                    "p (h c) -> p h c", h=2)
                nc.scalar.activation(dv, pc[:, :, :CCH], ACTF.Sqrt,
                                     bias=1.0, scale=-1.0,
                                     accum_out=rs[:, cc:cc + 1])

            def main_tail(i, rs, dq):
                """Row-mean affine + bf16 store for tile i."""
                rsum = norm_pool.tile([128, 1], F32, tag="rsum",
                                      name=f"rsum_{i}")
                bvec = norm_pool.tile([128, 1], F32, tag="bvec",
                                      name=f"bvec_{i}")
                nc.vector.reduce_sum(rsum[:, :], rs[:, :],
                                     axis=mybir.AxisListType.X)
                nc.vector.tensor_scalar(bvec[:, :], rsum[:, :], cb[:, 1:2],
                                        None, op0=ALU.mult)
                for q in range(NP2):
                    ob = ob_pool.tile([128, P2], BF16, tag="ob",
                                      name=f"ob_{i}_{q}")
                    nc.gpsimd.tensor_scalar(ob[:, :],
                                            dq[:, q * P2:(q + 1) * P2],
                                            cb[:, 0:1], bvec[:, 0:1],
                                            op0=ALU.mult, op1=ALU.add)
                    nc.sync.dma_start(
                        o_h[i * 128:(i + 1) * 128, q * P2:(q + 1) * P2],
                        ob[:, :])

            # prep interleaved with batch tile 0 so everything pipelines
            rs0 = rs_pool.tile([128, NBCH], F32, tag="rs", name="rs_0")
            dq0 = dq_pool.tile([128, C], BF16, tag="dq", name="dq_0")
            prep_segment(0)
            for s in range(1, NBCH + 1):
                prep_segment(s)
                main_chunk(0, s - 1, rs0, dq0)
            main_tail(0, rs0, dq0)
            for i in range(1, NB):
                rs = rs_pool.tile([128, NBCH], F32, tag="rs", name=f"rs_{i}")
                dq = dq_pool.tile([128, C], BF16, tag="dq", name=f"dq_{i}")
                for cc in range(NBCH):
                    main_chunk(i, cc, rs, dq)
                main_tail(i, rs, dq)

    nc.compile()
    return nc


_CACHE = {}


def _get_nc():
    if "nc" not in _CACHE:
        _CACHE["nc"] = build_nc()
    return _CACHE["nc"]


def make_in_maps(features, prototypes, distance_scale, temperature):
    f = np.asarray(features, dtype=np.float32)
    ft = np.ascontiguousarray(f.T)              # [D, B]
    pt = np.ascontiguousarray(
        np.asarray(prototypes, dtype=np.float32).T)  # [D, C]
    s = np.array([[np.float32(np.asarray(distance_scale).reshape(-1)[0]),
                   np.float32(np.asarray(temperature).reshape(-1)[0])]],
                 dtype=np.float32)
    return [
        {"ft": np.ascontiguousarray(ft[:, i * BPC:(i + 1) * BPC]),
         "pt": pt, "s": s}
        for i in range(N_CORES)
    ]


def run(features, prototypes, distance_scale, temperature, **kwargs):
    nc = _get_nc()
    in_maps = make_in_maps(features, prototypes, distance_scale, temperature)
    res = run_bass_kernel_spmd(nc, in_maps, core_ids=list(range(N_CORES)),
                               **kwargs)
    out = np.concatenate(
        [np.asarray(res.results[i]["o"]).astype(np.float32)
         for i in range(N_CORES)], axis=0)
    return out, res


def kernel(features, prototypes, distance_scale, temperature):
    out, _ = run(features, prototypes, distance_scale, temperature)
    return out

